# revision 16
# baseline (speedup 1.0000x reference)
"""Trainium2 Bass kernel for batched multi-head attention (v11, fp16).

Full module:  out = softmax((X_q Wq)(X_k Wk)^T / sqrt(dh) + keymask) (X_v Wv) * qmask
Shapes: B=4, S=2048, D=1024, H=16, dh=64.  Measured: ~393us (v2 baseline 457us).

Sharding over 8 NeuronCores: core c -> (batch b = c//2, head-group g = c%2).
Each core computes batch b, heads g*8..g*8+8 (Wq/Wk/Wv column-sharded by head).
Host-side prep (layout only): X pre-transposed to [D, S] fp16, W pre-chunked
to the on-chip staging layouts (strided-rearrange DMAs ran at ~100 GB/s in
256B packets and cost a 16us lead-in), and the final softmax divide +
q-mask + output transpose happen while unsharding: the device returns, per
(head, q-half), the UNNORMALIZED O^T = [VW|1]^T P as a [65, 1024] fp16
strip whose 65th row is the softmax denominator.

Design notes (what the profile said):
  - ACT is the hard floor: 256 x EXP[128,1024] at (N+352)/1.2ns = 285us.
    Exps of the two heads of a pair are STAGGERED (even head's KW/QW on
    partitions 0:64, odd on 64:128) so ACT never waits on the
    single-buffered S^T PSUM tiles.
  - The PE is the binding engine (~370us occupancy): S^T+AV streaming
    (~220us) + projections (~85us) + LDWEIGHTS (~half hidden).  v2 spent
    ~100us more on PE X-transposes and ~25us on tail O^T transposes --
    both eliminated via host-side layout work.
  - PSUM is exactly full (4 banks S^T + 4 banks O^T), so the projections
    are streamed INTO the attention iterations through two O^T-suspension
    windows per iteration: at kc=3/8 both heads' partial O^T fold into
    SBUF (DVE), the freed o-slots host the injected pp accumulations, and
    AV jobs run through budgeted per-head deques (p_pool buffers the P
    backlog) so injected matmuls never delay the S^T -> exp chain.
  - Schedule: serial prologue = K-mc0, Q-mc0 (+sh1), V sc8..15 (DMA-paced);
    (0,0) injects K-mc0-sh1 + V sc0..7; mc1..3 spread over later
    iterations, one group per window, finishing at (3,0).
  - Iteration boundaries are software-pipelined: each iteration's last AV
    drains + O^T evacuations defer into the next iteration's first exp
    window.  HAM stays at K=8/8 for the whole kernel (v2 lost ~60us to a
    mid-kernel 1.2 GHz window).
"""

import os
import sys
import time
import threading
from collections import deque

for _p in ("/opt/trn_rl_repo", "/opt/pypackages"):
    if _p not in sys.path and os.path.isdir(_p):
        sys.path.append(_p)

import numpy as np
from contextlib import ExitStack

import concourse.bass as bass
import concourse.tile as tile
from concourse import bacc, mybir
from concourse.bass_utils import run_bass_kernel_spmd

B, S, D = 4, 2048, 1024
HEADS, DH = 16, 64
NEG_BIG = 1e10
N_CORES = 8
HG = HEADS // 2          # 8 heads per core
MC = HG * DH             # 512 output cols per core
NSC = S // 128           # 16 seq chunks
NDC = D // 128           # 8 contraction chunks
NMC = MC // 128          # 4 head-dim chunks (of this core's 512 cols)
NKC = NSC                # 16 key chunks
NQH = 2                  # q halves
QH = S // NQH            # 1024

F32 = mybir.dt.float32
F16 = mybir.dt.float16
EXP = mybir.ActivationFunctionType.Exp
NP16 = np.float16

MM_N = 512               # fp16 moving-operand cap
NMM = QH // MM_N
AV_N = 512
NAV = QH // AV_N

SUSPA = 3                # first O^T segment boundary (injected iterations)
SUSPB = 8                # second boundary (two-window iterations)

# QK proj groups injected per iteration: two suspension windows, each a
# list of (kind, mcI, sh).  Deadline: a group is consumed from the NEXT
# iteration on.  ((0,0) instead injects the V sc0..7 groups; mc0 + q0sh1
# + V sc8..15 are projected in the serial prologue.)
INJ = {
    (0, 1): [[("k", 1, 0), ("k", 1, 1)], [("q", 1, 0)]],
    (1, 0): [[("q", 1, 1)], [("k", 2, 0)]],
    (1, 1): [[("k", 2, 1)], [("q", 2, 0)]],
    (2, 0): [[("q", 2, 1)], [("k", 3, 0)]],
    (2, 1): [[("k", 3, 1)], [("q", 3, 0)]],
    (3, 0): [[("q", 3, 1)]],
    (3, 1): [],
}


def _emit(tc, t):
    nc = tc.nc
    ctx = ExitStack()

    # ---------------- persistent pools ----------------
    cpool = ctx.enter_context(tc.tile_pool(name="const", bufs=1))
    vbias = cpool.tile([128, NKC], F32)

    qk_pool = ctx.enter_context(tc.tile_pool(name="qk", bufs=1))
    qwT = qk_pool.tile([128, NMC, S], F16)        # [m%128, mc, s]
    kwT = qk_pool.tile([128, NMC, S], F16)
    vw = qk_pool.tile([128, NKC, HG, DH + 1], F16)  # [k%128, kc, h, dh|1]
    ones = cpool.tile([128, 1], F32)
    nc.vector.memset(ones[:], 1.0)
    nc.vector.tensor_copy(                           # denominator ones column
        vw[:, :, :, DH:DH + 1], ones[:].broadcast_to([128, NKC, HG, 1])
    )

    # ---------------- weights + X^T staging ----------------
    # sync queue: weights; scalar queue: the X^T streams (ACT idle early).
    w_pool = ctx.enter_context(tc.tile_pool(name="w", bufs=1))
    w_qk = {}
    for kind in ("k", "q"):
        wt = w_pool.tile([128, NDC, NMC, 128], F16, name=f"w{kind}", tag=f"w{kind}")
        nc.sync.dma_start(wt[:], t["w" + kind].ap())
        w_qk[kind] = wt
    nc.sync.dma_start(vbias[:], t["vbias"].ap())
    x_pool = ctx.enter_context(tc.tile_pool(name="x", bufs=1))
    # xvA: X_v^T sh0 + Wv — consumed by (0,0)'s injected V proj, so it
    # must outlive the attention pools (freed only at the end).
    # xvB: X_v^T sh1 — prologue V proj only, freed before attention.
    xvA_pool = ctx.enter_context(tc.tile_pool(name="xvA", bufs=1))
    wv_sb = xvA_pool.tile([128, NDC, MC], F16, tag="wv")
    nc.sync.dma_start(wv_sb[:], t["wv"].ap())
    xvb_ctx = ExitStack()
    xvB_pool = xvb_ctx.enter_context(tc.tile_pool(name="xvB", bufs=1))
    xts = {}
    for xname, pool in (("xk", x_pool), ("xq", x_pool)):
        xts[xname] = pool.tile([128, NDC, S], F16, name=xname, tag=xname)
    xva = xvA_pool.tile([128, NDC, QH], F16, name="xva", tag="xva")
    xvb = xvB_pool.tile([128, NDC, QH], F16, name="xvb", tag="xvb")
    # consumption order: xk (K-mc0), xq sh0, xv sh1 (prologue V), xq sh1
    # (Q-mc0-sh1, last prologue group), xv sh0 ((0,0)'s injected V)
    for xname, sh in (("xk", 0), ("xq", 0), ("xv", 1), ("xv", 0),
                      ("xq", 1), ("xk", 1)):
        xdr = t[xname].ap().rearrange("(dc p) s -> dc p s", p=128)
        for dc in range(NDC):
            if xname == "xv":
                dst = (xva if sh == 0 else xvb)[:, dc, :]
            else:
                dst = xts[xname][:, dc, sh * QH:(sh + 1) * QH]
            nc.scalar.dma_start(dst, xdr[dc][:, sh * QH:(sh + 1) * QH])

    # ---------------- projection emitters ----------------
    def qk_half(pool, tag, kind, mcI, sh, nh):
        """One [128, 512] half of a QK proj group: 8 matmuls + CAST evac."""
        xt = xts["x" + kind]
        w_sb = w_qk[kind]
        pp = pool.tile([128, MM_N], F32, tag=tag,
                       name=f"pp_{kind}{mcI}{sh}{nh}")
        for dc in range(NDC):
            nc.tensor.matmul(
                pp[:],
                w_sb[:, dc, mcI, :],
                xt[:, dc, sh * QH + nh * MM_N:sh * QH + (nh + 1) * MM_N],
                start=(dc == 0),
                stop=(dc == NDC - 1),
            )
        dst = qwT if kind == "q" else kwT
        nc.vector.tensor_copy(
            dst[:, mcI, sh * QH + nh * MM_N:sh * QH + (nh + 1) * MM_N], pp[:]
        )

    def v_group(pool, tag, sc):
        """One V proj group: vw[:, sc] = X_v^T-chunk stationary x Wv moving."""
        xvh = xva if sc < 8 else xvb
        scl = sc % 8
        pv = pool.tile([128, MC], F32, tag=tag, name=f"pv{sc}")
        for dc in range(NDC):
            nc.tensor.matmul(
                pv[:],
                xvh[:, dc, scl * 128:(scl + 1) * 128],
                wv_sb[:, dc, :],
                start=(dc == 0),
                stop=(dc == NDC - 1),
            )
        nc.vector.tensor_copy(
            vw[:, sc, :, 0:DH], pv[:].rearrange("p (h d) -> p h d", h=HG)
        )

    # pre-warm the PE clock during the DMA lead-in: ~5us of dep-free tiny
    # matmuls so K-mc0 streams at 2.4 GHz the moment its data lands
    warm_ctx = ExitStack()
    warm_pool = warm_ctx.enter_context(
        tc.tile_pool(name="ps_w", bufs=1, space="PSUM"))
    scratch = cpool.tile([128, 64], F16)
    nc.vector.memset(scratch[:], 0.0)
    warm = warm_pool.tile([64, 64], F32, tag="warm")
    for i in range(100):
        nc.tensor.matmul(warm[:], scratch[:, 0:64], scratch[:, 0:64],
                         start=True, stop=True)
    warm_ctx.close()

    # ---------------- serial prologue ----------------
    # K-mc0 (both halves), Q-mc0-sh0, V sc8..15, Q-mc0-sh1; everything
    # else is injected into the attention stream below.
    pctx = ExitStack()
    psum_p = pctx.enter_context(tc.tile_pool(name="ps_p", bufs=2, space="PSUM"))
    for kind, mcI, sh in (("k", 0, 0), ("q", 0, 0)):
        for nh in range(NMM):
            qk_half(psum_p, "pp", kind, mcI, sh, nh)
    for sc in range(8, NSC):
        v_group(psum_p, "pp", sc)
    for sc in (6, 7):
        v_group(psum_p, "pp", sc)
    for nh in range(NMM):
        qk_half(psum_p, "pp", "q", 0, 1, nh)
    pctx.close()
    xvb_ctx.close()

    # ---------------- attention phase ----------------
    actx = ExitStack()
    p_pool = actx.enter_context(tc.tile_pool(name="p", bufs=19))
    ot_pool = actx.enter_context(tc.tile_pool(name="ot", bufs=4))
    psum_s = actx.enter_context(tc.tile_pool(name="ps_s", bufs=2, space="PSUM"))
    # ps_o / acc pools open lazily AFTER (0,0)'s V pv pool closes, so the
    # PSUM high-water mark stays at 8 banks (pool alloc is LIFO).
    o_state = {}
    acc_state = {}

    def psum_o():
        if "pool" not in o_state:
            es = ExitStack()
            o_state["ctx"] = es
            o_state["pool"] = es.enter_context(
                tc.tile_pool(name="ps_o", bufs=2, space="PSUM"))
        return o_state["pool"]

    def acc_pool():
        if "pool" not in acc_state:
            es = ExitStack()
            acc_state["ctx"] = es
            acc_state["pool"] = es.enter_context(
                tc.tile_pool(name="acc", bufs=2))
        return acc_state["pool"]

    out_v = t["out"].ap()
    finish = [None]   # previous iteration's deferred drain+evac closure

    for hp in range(HG // 2):
        mcI = hp
        kwh = (kwT[0:64, mcI, :], kwT[64:128, mcI, :])
        qwh = (qwT[0:64, mcI, :], qwT[64:128, mcI, :])
        for qh in range(NQH):
            q0 = qh * QH
            first_iter = hp == 0 and qh == 0
            windows = INJ.get((hp, qh), [])
            n_win = len(windows)

            s_t = [
                psum_s.tile([128, QH], F32, tag="s", name=f"s{i}_{hp}_{qh}")
                for i in range(2)
            ]

            def emit_S(i, kc, s_t=s_t, kwh=kwh, qwh=qwh, q0=q0):
                for nh in range(NMM):
                    nc.tensor.matmul(
                        s_t[i][:, nh * MM_N:(nh + 1) * MM_N],
                        kwh[i][:, kc * 128:(kc + 1) * 128],
                        qwh[i][:, q0 + nh * MM_N:q0 + (nh + 1) * MM_N],
                        start=True, stop=True,
                    )

            def emit_exp(i, kc, s_t=s_t, hp=hp, qh=qh):
                p_t = p_pool.tile([128, QH], F16, tag="p",
                                  name=f"p{i}_{hp}_{qh}_{kc}")
                nc.scalar.activation(
                    p_t[:], s_t[i][:], EXP,
                    bias=vbias[:, kc:kc + 1], scale=0.125,
                )
                return p_t

            # O^T segmentation state for suspension windows
            o_cur = [None, None]
            acc = [None, None]
            evacA = [0]
            evacB = [0]

            def emit_av(i, kc, p_t, o_cur=o_cur, acc=acc, evacA=evacA,
                        evacB=evacB, n_win=n_win, hp=hp, qh=qh):
                if o_cur[i] is None:
                    o_cur[i] = psum_o().tile(
                        [DH + 1, QH], F32, tag="o", name=f"o{i}_{hp}_{qh}_{kc}"
                    )
                first = kc == 0 or (n_win >= 1 and kc == SUSPA + 1) or (
                    n_win == 2 and kc == SUSPB + 1)
                last = kc == NKC - 1 or (n_win >= 1 and kc == SUSPA) or (
                    n_win == 2 and kc == SUSPB)
                for nh in range(NAV):
                    nc.tensor.matmul(
                        o_cur[i][:, nh * AV_N:(nh + 1) * AV_N],
                        vw[:, kc, 2 * hp + i, :],
                        p_t[:, nh * AV_N:(nh + 1) * AV_N],
                        start=first, stop=last,
                    )
                if n_win >= 1 and kc == SUSPA:
                    a = acc_pool().tile([DH + 1, QH], F32, tag="acc",
                                        name=f"acc{i}_{hp}_{qh}")
                    nc.vector.tensor_copy(a[:], o_cur[i][:])
                    acc[i] = a
                    o_cur[i] = None
                    evacA[0] += 1
                elif n_win == 2 and kc == SUSPB:
                    nc.vector.tensor_add(acc[i][:], acc[i][:], o_cur[i][:])
                    o_cur[i] = None
                    evacB[0] += 1

            # injected work windows
            if first_iter:
                vctx = ExitStack()
                vpool = vctx.enter_context(
                    tc.tile_pool(name="ps_v", bufs=2, space="PSUM"))
                injW = [
                    deque(
                        (lambda nh=nh: qk_half(vpool, "pv", "k", 0, 1, nh))
                        for nh in range(NMM)
                    ),
                    deque(
                        (lambda sc=sc: v_group(vpool, "pv", sc))
                        for sc in range(6)
                    ),
                ]
                injW[1].append(lambda: vctx.close())
            else:
                injW = []
                for win in windows:
                    wdq = deque()
                    for kind, mcI2, sh2 in win:
                        for nh in range(NMM):
                            wdq.append(
                                lambda kind=kind, mcI2=mcI2, sh2=sh2, nh=nh:
                                qk_half(psum_o(), "o", kind, mcI2, sh2, nh)
                            )
                    injW.append(wdq)
                while len(injW) < 2:
                    injW.append(deque())

            pend = (deque(), deque())

            def seg_ok(kc2, n_win=n_win, first_iter=first_iter, injW=injW):
                if first_iter:
                    # all AVs wait until the V/pp tiles vacate ps_v
                    return not (injW[0] or injW[1])
                if n_win == 0 or kc2 <= SUSPA:
                    return True
                if n_win == 1 or kc2 <= SUSPB:
                    return not injW[0]
                return not injW[1]

            pop_body = [-1]

            def sel_win(body, first_iter=first_iter, injW=injW,
                        evacA=evacA, evacB=evacB):
                if first_iter:
                    if injW[0]:
                        return injW[0]
                    return injW[1] if injW[1] else None
                if injW[0] and evacA[0] == 2 and body >= SUSPA + 3:
                    return injW[0]
                if (injW[1] and not injW[0] and evacB[0] == 2
                        and body >= SUSPB + 3):
                    return injW[1]
                return None

            def scan(pref, pend=pend, seg_ok=seg_ok):
                for i in (pref, 1 - pref):
                    if pend[i] and seg_ok(pend[i][0][1]):
                        return pend[i].popleft()
                return None

            def drain(pref, body, budget=None, pop_body=pop_body,
                      sel_win=sel_win, scan=scan, emit_av=emit_av,
                      pend=pend, injW=injW):
                if budget is not None:       # final flush: emit everything
                    while pend[0] or pend[1] or injW[0] or injW[1]:
                        w = sel_win(body)
                        if w:
                            w.popleft()()
                        job = scan(pref)
                        while job is not None:
                            emit_av(*job)
                            job = scan(pref)
                    return
                popped = 0
                job = scan(pref)
                if job is None:
                    w = sel_win(body)    # AVs gated: fill the PE with inj
                    if w:
                        w.popleft()()
                        popped += 1
                        job = scan(pref)
                if job is not None:
                    emit_av(*job)
                    if popped == 0:
                        job2 = scan(pref)
                        if job2 is not None:
                            emit_av(*job2)
                w = sel_win(body)
                if w and pop_body[0] != body and popped == 0:
                    w.popleft()()
                    pop_body[0] = body

            emit_S(0, 0)
            emit_S(1, 0)
            p0 = emit_exp(0, 0)
            # finish the previous iteration under this one's first exps
            if finish[0] is not None:
                finish[0]()
                finish[0] = None

            for kc in range(NKC):
                if kc > 0:
                    p0 = emit_exp(0, kc)
                pend[0].append((0, kc, p0))
                if kc + 1 < NKC:
                    emit_S(0, kc + 1)
                drain(1, kc)
                p1 = emit_exp(1, kc)
                pend[1].append((1, kc, p1))
                if kc + 1 < NKC:
                    emit_S(1, kc + 1)
                drain(0, kc)

            def make_finish(hp=hp, qh=qh, pend=pend, injW=injW, drain=drain,
                            o_cur=o_cur, acc=acc, n_win=n_win):
                def fin():
                    while pend[0] or pend[1] or injW[0] or injW[1]:
                        drain(0, NKC + 16, budget=100000)
                    for i in range(2):
                        ot = ot_pool.tile([DH + 1, QH], F16, tag="ot",
                                          name=f"ot_{2 * hp + i}_{qh}")
                        if n_win >= 1:
                            nc.vector.tensor_add(ot[:], acc[i][:],
                                                 o_cur[i][:])
                        else:
                            nc.vector.tensor_copy(ot[:], o_cur[i][:])
                        nc.sync.dma_start(out_v[2 * hp + i, qh], ot[:])
                return fin

            finish[0] = make_finish()

    finish[0]()
    if "ctx" in acc_state:
        acc_state["ctx"].close()
    if "ctx" in o_state:
        o_state["ctx"].close()
    actx.close()
    ctx.close()


_BUILD_LOCK = threading.Lock()
_CACHE = {}


def _build():
    with _BUILD_LOCK:
        if "nc" in _CACHE:
            return _CACHE["nc"]
        nc = bacc.Bacc(
            "TRN2", target_bir_lowering=False, debug=False, num_devices=N_CORES
        )
        t = {
            "xq": nc.dram_tensor("xq", [D, S], F16, kind="ExternalInput"),
            "xk": nc.dram_tensor("xk", [D, S], F16, kind="ExternalInput"),
            "xv": nc.dram_tensor("xv", [D, S], F16, kind="ExternalInput"),
            "wq": nc.dram_tensor("wq", [128, NDC, NMC, 128], F16,
                                 kind="ExternalInput"),
            "wk": nc.dram_tensor("wk", [128, NDC, NMC, 128], F16,
                                 kind="ExternalInput"),
            "wv": nc.dram_tensor("wv", [128, NDC, MC], F16,
                                 kind="ExternalInput"),
            "vbias": nc.dram_tensor("vbias", [128, NKC], F32, kind="ExternalInput"),
            "out": nc.dram_tensor("out", [HG, NQH, DH + 1, QH], F16,
                                  kind="ExternalOutput"),
        }
        with tile.TileContext(nc) as tc:
            _emit(tc, t)
        nc.compile()
        _CACHE["nc"] = nc
        return nc


def _in_maps(q_value, k_value, v_value, v_mask, Wq, Wk, Wv):
    maps = []
    x16 = {
        "xq": q_value.astype(NP16), "xk": k_value.astype(NP16),
        "xv": v_value.astype(NP16),
    }
    for c in range(N_CORES):
        b, g = c // 2, c % 2
        m0 = g * MC
        vb = ((v_mask[b, :, 0].reshape(NKC, 128).T) - 1.0) * NEG_BIG
        def wchunk(W):      # [D, MC] -> [p, dc, mc, m]
            return np.ascontiguousarray(
                W[:, m0:m0 + MC].astype(NP16)
                .reshape(NDC, 128, NMC, 128).transpose(1, 0, 2, 3))

        wv_c = np.ascontiguousarray(
            Wv[:, m0:m0 + MC].astype(NP16).reshape(NDC, 128, MC)
            .transpose(1, 0, 2))
        maps.append({
            "xq": np.ascontiguousarray(x16["xq"][b].T),
            "xk": np.ascontiguousarray(x16["xk"][b].T),
            "xv": np.ascontiguousarray(x16["xv"][b].T),
            "wq": wchunk(Wq),
            "wk": wchunk(Wk),
            "wv": wv_c,
            "vbias": np.ascontiguousarray(vb).astype(np.float32),
        })
    return maps


def _assemble(results, q_mask):
    out = np.empty((B, S, HEADS * DH), dtype=np.float32)
    qm = q_mask[:, :, 0].astype(np.float32)          # [B, S]
    for c in range(N_CORES):
        b, g = c // 2, c % 2
        o = results[c]["out"].astype(np.float32)     # [HG, NQH, 65, QH]
        num = o[:, :, :DH, :]                        # [HG, NQH, 64, QH]
        den = o[:, :, DH, :]                         # [HG, NQH, QH]
        norm = num / den[:, :, None, :]              # [HG, NQH, 64, QH]
        # -> [S, HG*64]: q index = qh*QH + q, col = h*64 + d
        blk = norm.transpose(1, 3, 0, 2).reshape(S, MC)
        out[b, :, g * MC:(g + 1) * MC] = blk * qm[b][:, None]
    return out


def kernel(q_value, k_value, v_value, v_mask, q_mask, Wq, Wk, Wv,
           profile=False, trace_cores=None):
    nc = _build()
    q_mask = np.asarray(q_mask, dtype=np.float32)
    maps = _in_maps(np.asarray(q_value, dtype=np.float32),
                    np.asarray(k_value, dtype=np.float32),
                    np.asarray(v_value, dtype=np.float32),
                    np.asarray(v_mask, dtype=np.float32),
                    np.asarray(Wq, dtype=np.float32),
                    np.asarray(Wk, dtype=np.float32),
                    np.asarray(Wv, dtype=np.float32))
    if profile:
        _install_profile_hook()
    res = run_bass_kernel_spmd(
        nc, maps, list(range(N_CORES)),
        trace=profile, trace_cores=trace_cores,
    )
    out = _assemble(res.results, q_mask)
    if profile:
        return out, res
    return out


def _install_profile_hook():
    """Wire up the NTFF profile hook that this container image lacks."""
    import types
    if "antenv.axon_hooks" in sys.modules:
        return
    try:
        from trn_agent_boot.trn_boot import _ntff_profile_via_ctypes
        hook = _ntff_profile_via_ctypes("/opt/axon/libaxon_pjrt.so")
    except Exception:
        hook = None
    mod = types.ModuleType("antenv.axon_hooks")
    mod.get_axon_ntff_profile_hook = lambda: hook
    sys.modules["antenv.axon_hooks"] = mod


if __name__ == "__main__":
    t0 = time.time()
    _build()
    print(f"build+compile: {time.time() - t0:.1f}s")
